# revision 4
# baseline (speedup 1.0000x reference)
"""NT-Xent loss on 8 Trainium2 NeuronCores (Bass/Tile SPMD kernel).

Math (matches the jax reference):
  z  = concat(z_i, z_j)                # [8192, 512]
  zn = z / ||z||                       # cosine-normalized rows
  sim = (zn @ zn.T) / T                # [8192, 8192]
  denom_i = sum_j!=i exp(sim_ij)       # masked row sums
  pos_i   = sim[i, i+-B]
  loss = mean(log(denom_i + 1e-8) - pos_i)

Device strategy (8-way row sharding):
  - host: normalize z (fp32), scale by S=16, cast fp8e4m3, lay out
    transposed in DoubleRow order; each core gets the full matrix as the
    moving operand and its own 1024-row slice as the stationary operand.
  - core c: computes its [1024, 8192] tile of zn@zn.T on the PE array
    (fp8 DoubleRow, K=512 in 2 passes), fused exp+row-sum on the scalar
    engine (accum_out), diagonal self-sim removed analytically by
    subtracting exp(1/T) (no masking needed: cos(i,i)=1 exactly), then
    log(denom + 1e-8 - e^2) on-chip.  Positives are computed separately
    as fp32 pair dots on the vector engine.
  - host: sums 8x[128,8] log tiles + 8x[128,4] pair tiles -> scalar loss.
"""

import numpy as np
import ml_dtypes

import concourse.bacc as bacc
import concourse.bass as bass
import concourse.mybir as mybir
import concourse.tile as tile
from concourse.bass_utils import run_bass_kernel_spmd

# problem constants (hardcoded per contract)
B = 4096
TWO_B = 2 * B          # 8192 rows of z
D = 512                # feature dim
T = 0.5                # temperature
NCORES = 8
ROWS_PER_CORE = TWO_B // NCORES   # 1024
M_TILES = ROWS_PER_CORE // 128    # 8
N_REGIONS = 4                     # psum regions of 2048 per row tile
REGION = TWO_B // N_REGIONS       # 2048
FP8_SCALE = 16.0                  # pre-scale before e4m3 cast
EXP_SCALE = 1.0 / (FP8_SCALE * FP8_SCALE * T)  # psum -> sim/T
LN_BIAS = float(1e-8 - np.exp(1.0 / T))        # +eps, -self-sim
PAIRS_PER_CORE = B // NCORES      # 512
F8 = mybir.dt.float8e4
F32 = mybir.dt.float32
NP_F8 = ml_dtypes.float8_e4m3


def _build_nc():
    nc = bacc.Bacc("TRN2", target_bir_lowering=False, debug=False)

    # DoubleRow layout: element [c, p, j, col] = zq[col, 256*c + 128*j + p]
    znT8_d = nc.dram_tensor("znT8", [2, 128, 2, TWO_B], F8, kind="ExternalInput")
    myT8_d = nc.dram_tensor("myT8", [2, 128, 2, ROWS_PER_CORE], F8, kind="ExternalInput")
    # fp32 row-major pair slices: [t, p, d] = zn[base + 128*t + p, d]
    pairA_d = nc.dram_tensor("pairA", [4, 128, D], F32, kind="ExternalInput")
    pairB_d = nc.dram_tensor("pairB", [4, 128, D], F32, kind="ExternalInput")
    logs_d = nc.dram_tensor("logs", [128, M_TILES], F32, kind="ExternalOutput")
    pairs_d = nc.dram_tensor("pairs", [128, 4], F32, kind="ExternalOutput")

    with tile.TileContext(nc) as tc:
        with (
            tc.tile_pool(name="zn", bufs=1) as zn_pool,
            tc.tile_pool(name="my", bufs=1) as my_pool,
            tc.tile_pool(name="pair", bufs=1) as pair_pool,
            tc.tile_pool(name="small", bufs=1) as small_pool,
            tc.tile_pool(name="psum", bufs=2, space=bass.MemorySpace.PSUM) as psum_pool,
            tc.tile_pool(name="scr", bufs=2) as scr_pool,
        ):
            zt = zn_pool.tile([128, 2, 2, TWO_B], F8)
            myt = my_pool.tile([128, 2, 2, ROWS_PER_CORE], F8)
            pa = pair_pool.tile([128, 4, D], F32)
            pb = pair_pool.tile([128, 4, D], F32)
            acc = small_pool.tile([128, M_TILES * N_REGIONS], F32)
            denoms = small_pool.tile([128, M_TILES], F32)
            logs_t = small_pool.tile([128, M_TILES], F32)
            pair_acc = small_pool.tile([128, 4], F32)
            pair_scr = small_pool.tile([128, D], F32)
            ln_bias = small_pool.tile([128, 1], F32)
            nc.gpsimd.memset(ln_bias[:], LN_BIAS)

            # loads; zt split by n-region so first matmuls start early
            for c in range(2):
                for r in range(N_REGIONS):
                    nc.sync.dma_start(
                        zt[:, c, :, r * REGION:(r + 1) * REGION],
                        znT8_d[c][:, :, r * REGION:(r + 1) * REGION],
                    )
                nc.sync.dma_start(myt[:, c], myT8_d[c])
            nc.sync.dma_start(pa[:], pairA_d.ap().rearrange("t p d -> p t d"))
            nc.sync.dma_start(pb[:], pairB_d.ap().rearrange("t p d -> p t d"))

            for m in range(M_TILES):
                for r in range(N_REGIONS):
                    ps = psum_pool.tile([128, REGION], F32)
                    for c in range(2):
                        w = myt[:, c, :, m * 128:(m + 1) * 128]  # [128,2,128]
                        for ns in range(4):
                            n0 = r * REGION + ns * 512
                            nc.tensor.matmul(
                                ps[:, ns * 512:(ns + 1) * 512],
                                w,
                                zt[:, c, :, n0:n0 + 512],
                                start=(c == 0),
                                stop=(c == 1),
                                perf_mode=mybir.MatmulPerfMode.DoubleRow,
                            )
                    sc = scr_pool.tile([128, REGION], F32)
                    idx = m * N_REGIONS + r
                    nc.scalar.activation(
                        sc[:], ps[:],
                        mybir.ActivationFunctionType.Exp,
                        scale=EXP_SCALE,
                        accum_out=acc[:, idx:idx + 1],
                    )

            # denom per row = sum of the 4 region sums; then log(denom+eps-e^2)
            nc.vector.reduce_sum(
                denoms[:].rearrange("p (m o) -> p m o", o=1),
                acc[:].rearrange("p (m r) -> p m r", r=N_REGIONS),
                axis=mybir.AxisListType.X,
            )
            nc.scalar.activation(
                logs_t[:], denoms[:],
                mybir.ActivationFunctionType.Ln,
                bias=ln_bias[:],
            )
            nc.sync.dma_start(logs_d.ap(), logs_t[:])

            # positives: fp32 pair dots, accum over D per partition
            for t in range(4):
                nc.vector.scalar_tensor_tensor(
                    pair_scr[:],
                    pa[:, t],
                    1.0,
                    pb[:, t],
                    op0=mybir.AluOpType.mult,
                    op1=mybir.AluOpType.mult,
                    accum_out=pair_acc[:, t:t + 1],
                )
            nc.sync.dma_start(pairs_d.ap(), pair_acc[:])

    nc.compile()
    return nc


_CACHE = {}


def _get_nc():
    if "nc" not in _CACHE:
        _CACHE["nc"] = _build_nc()
    return _CACHE["nc"]


def make_inputs(z_i: np.ndarray, z_j: np.ndarray):
    """Host-side shard prep: normalize, fp8-quantize, DoubleRow layout."""
    z = np.concatenate([np.asarray(z_i), np.asarray(z_j)], axis=0).astype(np.float32)
    norms = np.sqrt((z * z).sum(axis=1, dtype=np.float32))
    zn = z / np.maximum(norms, 1e-8)[:, None]          # [8192, 512] f32
    zq = (zn * FP8_SCALE).astype(NP_F8)                # fp8 rows
    # transposed DoubleRow layout [c, p, j, col]: d = 256c + 128j + p
    znT8 = np.ascontiguousarray(
        zq.T.reshape(2, 2, 128, TWO_B).transpose(0, 2, 1, 3)
    )
    in_maps = []
    for core in range(NCORES):
        r0 = core * ROWS_PER_CORE
        myT8 = np.ascontiguousarray(znT8[:, :, :, r0:r0 + ROWS_PER_CORE])
        p0 = core * PAIRS_PER_CORE
        pairA = np.ascontiguousarray(
            zn[p0:p0 + PAIRS_PER_CORE].reshape(4, 128, D))
        pairB = np.ascontiguousarray(
            zn[B + p0:B + p0 + PAIRS_PER_CORE].reshape(4, 128, D))
        in_maps.append(
            {"znT8": znT8, "myT8": myT8, "pairA": pairA, "pairB": pairB})
    return in_maps


def finish(results) -> np.ndarray:
    """Host-side unshard: combine per-core partials into the scalar loss."""
    logs_sum = 0.0
    pair_sum = 0.0
    for res in results:
        logs_sum += res["logs"].astype(np.float64).sum()
        pair_sum += res["pairs"].astype(np.float64).sum()
    loss = (logs_sum - (2.0 / T) * pair_sum) / TWO_B
    return np.float32(loss)


def kernel(z_i: np.ndarray, z_j: np.ndarray) -> np.ndarray:
    nc = _get_nc()
    in_maps = make_inputs(z_i, z_j)
    res = run_bass_kernel_spmd(nc, in_maps, list(range(NCORES)))
    return finish(res.results)


if __name__ == "__main__":
    rng = np.random.default_rng(0)
    out = kernel(
        rng.standard_normal((B, D), dtype=np.float32),
        rng.standard_normal((B, D), dtype=np.float32),
    )
    print("loss:", out)


# revision 6
# speedup vs baseline: 117.9538x; 117.9538x over previous
"""NT-Xent loss, V3: symmetric block-circulant sharding (computes 53% of the
similarity matrix instead of 100%).

Decomposition (validated): 16 row-panels of 512.  Panel r<8 computes column
blocks (r+q)%16 for q in [0,8]; panel r>=8 for q in [0,7].  Row sums cover a
row's q-band; the q>=1 blocks' COLUMN sums supply the remaining entries of
other rows' denominators (symmetry).  Self-sim removed on host (-e^{1/T}).
Core c owns panels {c, c+8} -> uniform 17 blocks/core (SPMD-safe).  Inputs are
column-ROTATED per core so all programs use identical offsets.

Engines: PE fp8-DoubleRow matmuls + a few all-ones colsum matmuls; ACT fused
exp+rowsum; GPSIMD partition_all_reduce for the wide colsums; DVE psum->sbuf
slab copies + pair dots.  Final log + assembly on host (needs cross-core
colsum combine anyway).
"""

import numpy as np
import ml_dtypes

import concourse.bacc as bacc
import concourse.bass as bass
import concourse.mybir as mybir
import concourse.tile as tile
from concourse import bass_isa
from concourse.bass_utils import run_bass_kernel_spmd

B = 4096
TWO_B = 2 * B
D = 512
T = 0.5
NCORES = 8
PANEL = 512
FP8_SCALE = 16.0
EXP_SCALE = 1.0 / (FP8_SCALE * FP8_SCALE * T)
SELF_SIM = float(np.exp(1.0 / T))
PAIRS_PER_CORE = B // NCORES
F8 = mybir.dt.float8e4
F32 = mybir.dt.float32
NP_F8 = ml_dtypes.float8_e4m3

# per-panel geometry in ROTATED column coords (identical on every core).
# regions: (roff, rwidth); eligible colsum slice per region: (eoff, ewidth, engine)
PANEL_A = {
    "msubs": range(0, 4),           # myT8 cols [0,512) = global rows 512c..
    "regions": [
        (0,    1536, (512,  1024, "pe")),    # q0..q2 ; colsum q1,q2
        (1536, 1536, (1536, 1536, "pool")),  # q3..q5
        (3072, 1536, (3072, 1536, "pool")),  # q6..q8 (q8 colsum-eligible, r<8)
    ],
}
PANEL_B = {
    "msubs": range(4, 8),           # myT8 cols [512,1024) = rows 4096+512c..
    "regions": [
        (4096, 1536, (4608, 1024, "pe")),    # q0..q2 ; colsum q1,q2
        (5632, 1536, (5632, 1536, "pool")),  # q3..q5
        (7168, 1024, (7168, 1024, "pe")),    # q6,q7
    ],
}
COLA_W = 1024 + 1536 + 1536   # 4096 per A-msub
COLB_W = 1024 + 1536 + 1024   # 3584 per B-msub


def _build_nc(repeats: int = 1):
    """repeats>1 builds a timing variant: the full body (input DMAs included)
    is emitted N times so (t(R2)-t(R1))/(R2-R1) isolates per-iteration HW time
    from launch overhead.  Outputs are simply rewritten each rep."""
    nc = bacc.Bacc("TRN2", target_bir_lowering=False, debug=False)

    zt_d = nc.dram_tensor("zt8", [2, 128, 2, TWO_B], F8, kind="ExternalInput")
    my_d = nc.dram_tensor("myT8", [2, 128, 2, 1024], F8, kind="ExternalInput")
    pairA_d = nc.dram_tensor("pairA", [4, 128, D], F32, kind="ExternalInput")
    pairB_d = nc.dram_tensor("pairB", [4, 128, D], F32, kind="ExternalInput")
    rows_d = nc.dram_tensor("rows", [128, 24], F32, kind="ExternalOutput")
    colA_d = nc.dram_tensor("colA", [4, COLA_W], F32, kind="ExternalOutput")
    colB_d = nc.dram_tensor("colB", [4, COLB_W], F32, kind="ExternalOutput")
    pairs_d = nc.dram_tensor("pairs", [128, 4], F32, kind="ExternalOutput")

    with tile.TileContext(nc) as tc:
        with (
            tc.tile_pool(name="zn", bufs=1) as zn_pool,
            tc.tile_pool(name="my", bufs=1) as my_pool,
            tc.tile_pool(name="pair", bufs=1) as pair_pool,
            tc.tile_pool(name="small", bufs=1) as small_pool,
            tc.tile_pool(name="psum", bufs=2, space=bass.MemorySpace.PSUM) as psum_pool,
            tc.tile_pool(name="cspsum", bufs=2, space=bass.MemorySpace.PSUM) as cs_psum_pool,
            tc.tile_pool(name="scr", bufs=7) as scr_pool,
            tc.tile_pool(name="csr", bufs=4) as csr_pool,
            tc.tile_pool(name="slab", bufs=4) as slab_pool,
        ):
            zt = zn_pool.tile([128, 2, 2, TWO_B], F8)
            myt = my_pool.tile([128, 2, 2, 1024], F8)
            pa = pair_pool.tile([128, 4, D], F32)
            pb = pair_pool.tile([128, 4, D], F32)
            rows_acc = small_pool.tile([128, 24], F32)
            pair_acc = small_pool.tile([128, 4], F32)
            pair_scr = small_pool.tile([128, D], F32)
            ones_w = small_pool.tile([128, 128], F32)
            nc.gpsimd.memset(ones_w[:], 1.0)

            def emit_colsum(col_d, msub_local, out_off, sc, soff, ew, eng):
                if eng == "off":
                    return
                if eng == "pool":
                    csr = csr_pool.tile([128, 1536], F32, tag="csr")
                    nc.gpsimd.partition_all_reduce(
                        csr[:, :ew], sc[:, soff:soff + ew],
                        channels=128, reduce_op=bass_isa.ReduceOp.add,
                    )
                    nc.sync.dma_start(
                        col_d[msub_local][out_off:out_off + ew], csr[:1, :ew])
                else:  # 'pe': all-ones matmul per 512 block
                    slab = slab_pool.tile([128, 1536], F32, tag="slab")
                    for bi in range(ew // 512):
                        cps = cs_psum_pool.tile([128, 512], F32, tag="cs")
                        nc.tensor.matmul(
                            cps[:], ones_w[:],
                            sc[:, soff + bi * 512: soff + (bi + 1) * 512],
                        )
                        nc.vector.tensor_copy(
                            slab[:, bi * 512:(bi + 1) * 512], cps[:])
                    nc.sync.dma_start(
                        col_d[msub_local][out_off:out_off + ew], slab[:1, :ew])

            # colsum ops for msub m are emitted after msub m+1's matmul+exp so
            # the in-order PE stream never waits on ACT (1-msub software pipe).
            for _rep in range(repeats):
                for c in range(2):
                    nc.sync.dma_start(myt[:, c], my_d[c])
                for q in range(4):
                    for c in range(2):
                        nc.sync.dma_start(
                            zt[:, c, :, q * 2048:(q + 1) * 2048],
                            zt_d[c][:, :, q * 2048:(q + 1) * 2048],
                        )
                nc.sync.dma_start(pa[:], pairA_d.ap().rearrange("t p d -> p t d"))
                nc.sync.dma_start(pb[:], pairB_d.ap().rearrange("t p d -> p t d"))

                pending = []
                work = [(panel, col_d, gm)
                        for panel, col_d in ((PANEL_A, colA_d), (PANEL_B, colB_d))
                        for gm in panel["msubs"]]
                for panel, col_d, gm in work:
                    out_off = 0
                    msub_local = gm % 4
                    new_pending = []
                    for ri, (roff, rw, (eoff, ew, eng)) in enumerate(panel["regions"]):
                        ps = psum_pool.tile([128, 1536], F32, tag="ps")
                        for c in range(2):
                            w = myt[:, c, :, gm * 128:(gm + 1) * 128]
                            for nb in range(rw // 512):
                                n0 = roff + nb * 512
                                nc.tensor.matmul(
                                    ps[:, nb * 512:(nb + 1) * 512],
                                    w,
                                    zt[:, c, :, n0:n0 + 512],
                                    start=(c == 0),
                                    stop=(c == 1),
                                    perf_mode=mybir.MatmulPerfMode.DoubleRow,
                                )
                        sc = scr_pool.tile([128, 1536], F32, tag="sc")
                        idx = gm * 3 + ri
                        nc.scalar.activation(
                            sc[:, :rw], ps[:, :rw],
                            mybir.ActivationFunctionType.Exp,
                            scale=EXP_SCALE,
                            accum_out=rows_acc[:, idx:idx + 1],
                        )
                        new_pending.append(
                            (col_d, msub_local, out_off, sc, eoff - roff, ew, eng))
                        out_off += ew
                    for args in pending:
                        emit_colsum(*args)
                    pending = new_pending
                for args in pending:
                    emit_colsum(*args)

                nc.sync.dma_start(rows_d.ap(), rows_acc[:])

                for t in range(4):
                    nc.vector.scalar_tensor_tensor(
                        pair_scr[:], pa[:, t], 1.0, pb[:, t],
                        op0=mybir.AluOpType.mult, op1=mybir.AluOpType.mult,
                        accum_out=pair_acc[:, t:t + 1],
                    )
                nc.sync.dma_start(pairs_d.ap(), pair_acc[:])

    nc.compile()
    return nc


_CACHE = {}


def _get_nc():
    if "nc" not in _CACHE:
        _CACHE["nc"] = _build_nc()
    return _CACHE["nc"]


def _quantize(z_i, z_j):
    z = np.concatenate([np.asarray(z_i), np.asarray(z_j)], axis=0).astype(np.float32)
    norms = np.sqrt((z * z).sum(axis=1, dtype=np.float32))
    zn = z / np.maximum(norms, 1e-8)[:, None]
    zq = (zn * FP8_SCALE).astype(NP_F8)      # [8192, 512] fp8 rows
    return zn, zq


def _dr_layout(zq_cols):
    """[rows(=512 dims), cols] fp8 -> DoubleRow [c, p, j, col]; d=256c+128j+p."""
    return np.ascontiguousarray(
        zq_cols.reshape(2, 2, 128, zq_cols.shape[1]).transpose(0, 2, 1, 3))


def make_inputs(z_i, z_j):
    zn, zq = _quantize(z_i, z_j)
    zqT = zq.T                                 # [512, 8192], d-major
    in_maps = []
    for core in range(NCORES):
        rot = np.roll(zqT, -PANEL * core, axis=1)    # rotated cols
        zt8 = _dr_layout(rot)
        rows = np.concatenate([
            zqT[:, PANEL * core: PANEL * (core + 1)],
            zqT[:, B + PANEL * core: B + PANEL * (core + 1)],
        ], axis=1)                              # [512, 1024] panels {c, c+8}
        my8 = _dr_layout(rows)
        p0 = core * PAIRS_PER_CORE
        pairA = np.ascontiguousarray(zn[p0:p0 + PAIRS_PER_CORE].reshape(4, 128, D))
        pairB = np.ascontiguousarray(zn[B + p0:B + p0 + PAIRS_PER_CORE].reshape(4, 128, D))
        in_maps.append({"zt8": zt8, "myT8": my8, "pairA": pairA, "pairB": pairB})
    return in_maps


def finish(results) -> np.ndarray:
    denom = np.zeros(TWO_B, dtype=np.float64)
    pair_sum = 0.0
    for core, res in enumerate(results):
        rows = res["rows"].astype(np.float64)        # [128, 24]
        for gm in range(8):
            rsum = rows[:, gm * 3: gm * 3 + 3].sum(1)    # [128]
            base = (PANEL * core if gm < 4 else B + PANEL * core) + 128 * (gm % 4)
            denom[base:base + 128] += rsum
        for name, panel in (("colA", PANEL_A), ("colB", PANEL_B)):
            col = res[name].astype(np.float64)       # [4, W]
            for mi in range(4):
                off = 0
                for roff, rw, (eoff, ew, eng) in panel["regions"]:
                    for b0 in range(0, ew, PANEL):  # 512-blocks never wrap
                        g0 = (PANEL * core + eoff + b0) % TWO_B
                        denom[g0:g0 + PANEL] += col[mi, off + b0:off + b0 + PANEL]
                    off += ew
        pair_sum += res["pairs"].astype(np.float64).sum()
    denom -= SELF_SIM
    logs = np.log(denom + 1e-8)
    loss = (logs.sum() - (2.0 / T) * pair_sum) / TWO_B
    return np.array(loss, dtype=np.float32)


def kernel(z_i: np.ndarray, z_j: np.ndarray) -> np.ndarray:
    nc = _get_nc()
    in_maps = make_inputs(z_i, z_j)
    res = run_bass_kernel_spmd(nc, in_maps, list(range(NCORES)))
    return finish(res.results)


# ---------- numpy model of one core's outputs (for CoreSim checks) ----------

def expected_core_outputs(in_maps, core):
    m = in_maps[core]
    zt = m["zt8"].astype(np.float32).transpose(0, 2, 1, 3).reshape(512, TWO_B)
    my = m["myT8"].astype(np.float32).transpose(0, 2, 1, 3).reshape(512, 1024)
    rows = np.zeros((128, 24), dtype=np.float32)
    colA = np.zeros((4, COLA_W), dtype=np.float32)
    colB = np.zeros((4, COLB_W), dtype=np.float32)
    for panel, col in ((PANEL_A, colA), (PANEL_B, colB)):
        for gm in panel["msubs"]:
            w = my[:, gm * 128:(gm + 1) * 128]
            off = 0
            for ri, (roff, rw, (eoff, ew, eng)) in enumerate(panel["regions"]):
                dots = w.T @ zt[:, roff:roff + rw]
                e = np.exp(dots.astype(np.float32) * np.float32(EXP_SCALE))
                rows[:, gm * 3 + ri] = e.sum(1, dtype=np.float32)
                soff = eoff - roff
                col[gm % 4, off:off + ew] = e[:, soff:soff + ew].sum(0, dtype=np.float32)
                off += ew
    pairs = (m["pairA"].astype(np.float64) * m["pairB"].astype(np.float64)).sum(2).T
    return {"rows": rows, "colA": colA, "colB": colB, "pairs": pairs}


if __name__ == "__main__":
    rng = np.random.default_rng(0)
    z_i = rng.standard_normal((B, D), dtype=np.float32)
    z_j = rng.standard_normal((B, D), dtype=np.float32)
    # host-model end-to-end check (no device)
    in_maps = make_inputs(z_i, z_j)
    fake = [expected_core_outputs(in_maps, c) for c in range(NCORES)]
    loss_model = finish(fake)
    # float64 reference
    z = np.concatenate([z_i, z_j], 0).astype(np.float64)
    n = np.linalg.norm(z, axis=-1)
    sim = (z @ z.T) / np.maximum(n[:, None] * n[None, :], 1e-8) / T
    pos = np.concatenate([np.diagonal(sim, B), np.diagonal(sim, -B)])
    dn = ((1.0 - np.eye(TWO_B)) * np.exp(sim)).sum(1)
    ref = np.mean(np.log(dn + 1e-8) - pos)
    print(f"model={loss_model:.7f} ref={ref:.7f} rel={abs(loss_model-ref)/abs(ref):.3e}")
